# revision 8
# baseline (speedup 1.0000x reference)
"""Trainium2 Bass kernel for the BEMv13 MoE-LoRA module.

Computation (per token t, full problem):
  base  = x @ W_base.T + b_base
  w     = softmax(x @ W_router + b_router)        # E=2 experts
  H     = x @ A_cat.T                             # [T, 16] LoRA down-proj, both experts
  G     = H * w_broadcast * (alpha/rank)          # per-expert routing weight
  out   = base + G @ B_cat.T

Sharding: tokens (batch*seq = 16384) split evenly across 8 NeuronCores;
all weights replicated. No cross-core communication.

On-core algorithm (per core, 2048 tokens, one 128-token tile at a time),
all matmul operands in bf16 (abs tolerance is ~1e-1; bf16 error ~1e-2):
  - x is pre-transposed AND pre-tiled on the host to [tile, d, tok] bf16
    so no on-chip transposes or casts are needed at all.
  - W^T [D, O] bf16 is streamed in once and stays resident in SBUF
    (64 KB/partition).
  - Main matmul: out[128 tok, 512 o] accumulated over 16 k-tiles in PSUM
    (5 rotating banks for the 4 accumulators); stationary xt tile shared
    by the LoRA/router matmul (aat, 18 cols) of each k-group.
  - softmax over 2 experts == sigmoid of the logit difference.
  - G is extended with a ones column; B_cat^T with a b_base row, so the
    K=17 LoRA finalize matmul also adds the output bias -> the drain is a
    pure PSUM->SBUF bf16 copy, split between Vector and Scalar engines.
  - Output is written to HBM in bf16 and upcast on the host.
"""

import numpy as np
import ml_dtypes

BF16 = ml_dtypes.bfloat16

P = 128
D = 2048
O = 2048
KT = D // P            # 16 k-tiles
NT = 16                # 128-token tiles per core
TOK = 2048             # tokens per core
HN = 18                # 16 LoRA cols + 1 router-diff col + 1 pad
ER = 16                # E*R
GK = 18                # LoRA finalize: 16 lora rows + 1 bias/ones row + 1 pad
SCALE = 16.0 / 8.0
NCORES = 8

_CACHE = {}


def _build():
    import concourse.tile as tile
    import concourse.masks as masks
    from concourse import bacc, mybir

    f32 = mybir.dt.float32
    bf16 = mybir.dt.bfloat16

    nc = bacc.Bacc("TRN2", target_bir_lowering=False, debug=False)

    # xt: host-pretransposed x, row block t*128+p holds, for col k*128+j,
    # the value x[t*128+j, k*128+p]  (d-major within each token tile)
    xt_d = nc.dram_tensor("xt", [NT * P, KT * P], bf16, kind="ExternalInput")
    wt_d = nc.dram_tensor("wt", [D, O], bf16, kind="ExternalInput")
    aat_d = nc.dram_tensor("aat", [P, KT * HN], bf16, kind="ExternalInput")
    btx_d = nc.dram_tensor("btx", [P, 512], bf16, kind="ExternalInput")
    brd_d = nc.dram_tensor("brd", [1, 1], f32, kind="ExternalInput")
    out_d = nc.dram_tensor("out", [TOK, O], bf16, kind="ExternalOutput")

    with tile.TileContext(nc) as tc:
        with (
            tc.tile_pool(name="res", bufs=1) as res,
            tc.tile_pool(name="xpool", bufs=4) as xpool,
            tc.tile_pool(name="opool", bufs=2) as opool,
            tc.tile_pool(name="gpool", bufs=2) as gpool,
            tc.tile_pool(name="small", bufs=2) as small,
            tc.tile_pool(name="psA", bufs=6, space="PSUM") as psA,
            tc.tile_pool(name="psH", bufs=1, space="PSUM") as psH,
            tc.tile_pool(name="psT", bufs=1, space="PSUM") as psT,
        ):
            # All startup-critical loads go on ONE queue (SP/sync) in
            # hand-chosen FIFO order: the first matmuls need aat+xt0, then
            # the W slabs pace tile 0, with xt1 slotted early. Later x tiles
            # and btx are triggered from the ACT stream *after* each tile's
            # sigmoid so they queue behind a data dependency and cannot
            # crowd the W stream.
            aat_sb = res.tile([P, KT * HN], bf16, tag="aat")
            btx_sb = res.tile([P, 512], bf16, tag="btx")
            brd128 = res.tile([P, 1], f32, tag="brd128")
            nc.gpsimd.dma_start(brd128[:], brd_d[:].broadcast_to((P, 1)))

            xts = [None] * NT

            def load_x(t, eng):
                xts[t] = xpool.tile([P, KT * P], bf16, tag="xt", name=f"xt_{t}")
                eng.dma_start(xts[t][:], xt_d[t * P:(t + 1) * P, :])

            w_sb = res.tile([P, KT * O], bf16, tag="w_sb")

            def load_w(k):
                nc.sync.dma_start(w_sb[:, k * O:(k + 1) * O],
                                  wt_d[k * P:(k + 1) * P, :])

            nc.sync.dma_start(aat_sb[:], aat_d[:])
            load_x(0, nc.sync)
            load_w(0)
            load_w(1)
            load_x(1, nc.sync)
            for k in range(2, KT):
                load_w(k)

            ident = res.tile([P, P], bf16, tag="ident")
            masks.make_identity(nc, ident[:])

            # --- main loop: one 128-token tile at a time.
            # LoRA finalize for tile t is emitted after tile t's main sweep;
            # the h->sigmoid->g->g^T chain latency is mostly hidden because
            # the h matmul leads each k-group and the chain engines (ACT/DVE)
            # are otherwise idle.
            for t in range(NT):

                accs = [
                    psA.tile([P, 512], f32, tag="acc", name=f"acc_{t}_{j}")
                    for j in range(4)
                ]
                h = psH.tile([P, HN], f32, tag="h", name=f"h_{t}")
                gt4 = gpool.tile([P, P], bf16, tag="gt4", name=f"gt4_{t}")
                for k in range(KT):
                    lhs = xts[t][:, k * P:(k + 1) * P]
                    # h matmuls 2-per-iteration (front-loaded so the routing
                    # chain finishes during the main sweep), interleaved
                    # after the 512-col matmuls so their LDWEIGHTS hide.
                    for j in range(4):
                        nc.tensor.matmul(
                            accs[j][:], lhs,
                            w_sb[:, k * O + j * 512:k * O + (j + 1) * 512],
                            start=(k == 0), stop=False,
                        )
                        if k < 8 and j < 2:
                            c = 2 * k + j
                            nc.tensor.matmul(
                                h[:], xts[t][:, c * P:(c + 1) * P],
                                aat_sb[:, c * HN:(c + 1) * HN],
                                start=(c == 0), stop=(c == KT - 1))
                    # HAM warmers: while the PE trails the W^T DMA stream in
                    # tile 0 it would idle between k-groups and re-throttle to
                    # half clock; re-reading the just-arrived slab keeps it
                    # warm and costs nothing once W is resident.
                    if t == 0:
                        wdum = psT.tile([P, 512], f32, tag="tstage",
                                        name=f"wd_{k}")
                        nc.tensor.matmul(wdum[:], lhs,
                                         w_sb[:, k * O:k * O + 512],
                                         start=True, stop=True)
                        nc.tensor.matmul(wdum[:], lhs,
                                         w_sb[:, k * O + 512:k * O + 1024],
                                         start=True, stop=True)
                    if k == 7:
                        # routing: w1 = sigmoid(dlogit + brd), w0 = 1 - w1,
                        # both scaled by alpha/rank
                        srow = small.tile([P, 1], f32, tag="srow",
                                          name=f"srow_{t}")
                        nc.scalar.activation(
                            srow[:], h[:, ER:ER + 1],
                            mybir.ActivationFunctionType.Sigmoid,
                            bias=brd128[:, 0:1], scale=1.0)
                        w1s = small.tile([P, 1], f32, tag="w1s", name=f"w1s_{t}")
                        nc.vector.tensor_scalar_mul(w1s[:], srow[:], SCALE)
                        w0s = small.tile([P, 1], f32, tag="w0s", name=f"w0s_{t}")
                        nc.vector.tensor_scalar(
                            w0s[:], srow[:], -SCALE, SCALE,
                            mybir.AluOpType.mult, mybir.AluOpType.add)
                        g = gpool.tile([P, GK], bf16, tag="g", name=f"g_{t}")
                        nc.vector.tensor_scalar_mul(g[:, 0:8], h[:, 0:8], w0s[:])
                        nc.vector.tensor_scalar_mul(g[:, 8:16], h[:, 8:16], w1s[:])
                        nc.vector.memset(g[:, 16:17], 1.0)
                        nc.vector.memset(g[:, 17:18], 0.0)
                    if k == 11:
                        # g^T on the PE, then replicate into 4 row strips so
                        # the finalize matmuls can run on 4 concurrent 32-row
                        # tiles of the array.
                        gst = psT.tile([GK, P], bf16, tag="tstage",
                                       name=f"gst_{t}")
                        nc.tensor.transpose(gst[:], g[:], ident[:])
                        for jj in range(4):
                            nc.scalar.copy(gt4[32 * jj:32 * jj + GK, :], gst[:])
                        # next-next x tile (and btx once): triggered here so
                        # the transfer queues behind tile t's progress instead
                        # of crowding the W stream at startup
                        if t == 0:
                            nc.scalar.dma_start(btx_sb[:], btx_d[:])
                        if t + 2 < NT:
                            load_x(t + 2, nc.scalar)

                # LoRA finalize + bias: 4 concurrent row-tiled K=18 matmuls
                for jj in range(4):
                    nc.tensor.matmul(accs[jj][:],
                                     gt4[32 * jj:32 * jj + GK, :],
                                     btx_sb[32 * jj:32 * jj + GK, :],
                                     start=False, stop=True,
                                     tile_position=(32 * jj, 0))

                # drain: bias already folded in via the ones-row; pure copy,
                # split across Vector and Scalar engines.
                outt = opool.tile([P, O], bf16, tag="outt", name=f"out_{t}")
                for j in range(4):
                    if j < 2:
                        nc.vector.tensor_copy(outt[:, j * 512:(j + 1) * 512],
                                              accs[j][:])
                    else:
                        nc.scalar.copy(outt[:, j * 512:(j + 1) * 512],
                                       accs[j][:])
                    if j % 2 == 1:
                        nc.sync.dma_start(
                            out_d[t * P:(t + 1) * P, (j - 1) * 512:(j + 1) * 512],
                            outt[:, (j - 1) * 512:(j + 1) * 512])

    nc.compile()
    return nc


def _prep_host(x, W_base, b_base, A, B, W_router, b_router):
    """Host-side layout prep + sharding. Returns per-core input maps."""
    x_flat = np.ascontiguousarray(np.asarray(x, dtype=np.float32).reshape(-1, D))
    xb = x_flat.astype(BF16)
    # per-core pre-transposed tiling: [core, t, p(d within k), k*128+j(tok)]
    xt_all = np.ascontiguousarray(
        xb.reshape(NCORES, NT, P, KT, P).transpose(0, 1, 4, 3, 2)
    ).reshape(NCORES, NT * P, KT * P)

    wt = np.ascontiguousarray(np.asarray(W_base, dtype=np.float32).T).astype(BF16)

    a_cat = np.asarray(A, dtype=np.float32).reshape(ER, D)          # [16, D]
    aat = np.zeros((D, HN), dtype=np.float32)
    aat[:, :ER] = a_cat.T
    wr = np.asarray(W_router, dtype=np.float32)
    aat[:, ER] = wr[:, 1] - wr[:, 0]
    # pre-arrange for contiguous per-partition DMA: [P, KT*HN]
    aat = np.ascontiguousarray(
        aat.reshape(KT, P, HN).transpose(1, 0, 2).reshape(P, KT * HN)
    ).astype(BF16)

    b_cat = np.concatenate([np.asarray(B, dtype=np.float32)[0],
                            np.asarray(B, dtype=np.float32)[1]], axis=1)  # [O, 16]
    btx_full = np.zeros((GK, O), dtype=np.float32)
    btx_full[:ER] = b_cat.T
    btx_full[ER] = np.asarray(b_base, dtype=np.float32)  # ones-row adds bias
    # 4 row strips at partition offsets 0/32/64/96, one 512-col chunk each,
    # so the finalize matmuls run on 4 concurrent 32-row PE tiles
    btx = np.zeros((P, 512), dtype=np.float32)
    for jj in range(4):
        btx[32 * jj:32 * jj + GK] = btx_full[:, jj * 512:(jj + 1) * 512]
    btx = np.ascontiguousarray(btx).astype(BF16)

    brd = np.array([[np.float32(b_router[1]) - np.float32(b_router[0])]],
                   dtype=np.float32)

    in_maps = []
    for c in range(NCORES):
        in_maps.append({
            "xt": xt_all[c],
            "wt": wt,
            "aat": aat,
            "btx": btx,
            "brd": brd,
        })
    return in_maps


def kernel(x, W_base, b_base, A, B, W_router, b_router):
    from concourse import bass_utils

    # NOTE: walrus's LDWEIGHTS-dedup (--enable-ldw-opt=true) rejects bf16
    # weight loads ("InstLdweights is not compatible with LDW optimization")
    # because they take the FWL path; bf16 LDWs are ~2x faster than fp32 and
    # hidden by the PE's pull-ahead window, so the dedup isn't needed.
    if "nc" not in _CACHE:
        _CACHE["nc"] = _build()
    nc = _CACHE["nc"]

    in_maps = _prep_host(x, W_base, b_base, A, B, W_router, b_router)
    res = None
    for attempt in range(3):
        try:
            res = bass_utils.run_bass_kernel_spmd(
                nc, in_maps, core_ids=list(range(NCORES)))
            break
        except Exception:
            # rare transient NRT_EXEC_UNIT_UNRECOVERABLE observed once;
            # the same NEFF runs fine on retry
            if attempt == 2:
                raise
    out = np.concatenate(
        [np.asarray(res.results[c]["out"]).astype(np.float32)
         for c in range(NCORES)], axis=0)
    return out.reshape(np.asarray(x).shape[0], -1, O)


# revision 9
# speedup vs baseline: 1.2031x; 1.2031x over previous
"""Trainium2 Bass kernel for the BEMv13 MoE-LoRA module.

Computation (per token t, full problem):
  base  = x @ W_base.T + b_base
  w     = softmax(x @ W_router + b_router)        # E=2 experts
  H     = x @ A_cat.T                             # [T, 16] LoRA down-proj, both experts
  G     = H * w_broadcast * (alpha/rank)          # per-expert routing weight
  out   = base + G @ B_cat.T

Sharding: tokens (batch*seq = 16384) split evenly across 8 NeuronCores;
all weights replicated. No cross-core communication.

On-core algorithm (per core, 2048 tokens, one 128-token tile at a time),
all matmul operands in bf16 (abs tolerance is ~1e-1; bf16 error ~1e-2):
  - x is pre-transposed AND pre-tiled on the host to [tile, d, tok] bf16
    so no on-chip transposes or casts are needed at all.
  - W^T [D, O] bf16 is streamed in once and stays resident in SBUF
    (64 KB/partition).
  - Main matmul: out[128 tok, 512 o] accumulated over 16 k-tiles in PSUM
    (5 rotating banks for the 4 accumulators); stationary xt tile shared
    by the LoRA/router matmul (aat, 18 cols) of each k-group.
  - softmax over 2 experts == sigmoid of the logit difference.
  - G is extended with a ones column; B_cat^T with a b_base row, so the
    K=17 LoRA finalize matmul also adds the output bias -> the drain is a
    pure PSUM->SBUF bf16 copy, split between Vector and Scalar engines.
  - Output is written to HBM in bf16 and upcast on the host.
"""

import numpy as np
import ml_dtypes

BF16 = ml_dtypes.bfloat16

P = 128
D = 2048
O = 2048
KT = D // P            # 16 k-tiles
NT = 16                # 128-token tiles per core
TOK = 2048             # tokens per core
HN = 18                # 16 LoRA cols + 1 router-diff col + 1 pad
ER = 16                # E*R
GK = 18                # LoRA finalize: 16 lora rows + 1 bias/ones row + 1 pad
SCALE = 16.0 / 8.0
NCORES = 8

_CACHE = {}


def _build():
    import concourse.tile as tile
    import concourse.masks as masks
    from concourse import bacc, mybir

    f32 = mybir.dt.float32
    bf16 = mybir.dt.bfloat16

    nc = bacc.Bacc("TRN2", target_bir_lowering=False, debug=False)

    # xt: host-pretransposed x, row block t*128+p holds, for col k*128+j,
    # the value x[t*128+j, k*128+p]  (d-major within each token tile)
    xt_d = nc.dram_tensor("xt", [NT * P, KT * P], bf16, kind="ExternalInput")
    wt_d = nc.dram_tensor("wt", [D, O], bf16, kind="ExternalInput")
    aat_d = nc.dram_tensor("aat", [P, KT * HN], bf16, kind="ExternalInput")
    btx_d = nc.dram_tensor("btx", [P, 512], bf16, kind="ExternalInput")
    brd_d = nc.dram_tensor("brd", [1, 1], f32, kind="ExternalInput")
    out_d = nc.dram_tensor("out", [TOK, O], bf16, kind="ExternalOutput")

    with tile.TileContext(nc) as tc:
        with (
            tc.tile_pool(name="res", bufs=1) as res,
            tc.tile_pool(name="xpool", bufs=2) as xpool,
            tc.tile_pool(name="opool", bufs=2) as opool,
            tc.tile_pool(name="gpool", bufs=2) as gpool,
            tc.tile_pool(name="small", bufs=2) as small,
            tc.tile_pool(name="psA", bufs=6, space="PSUM") as psA,
            tc.tile_pool(name="psH", bufs=1, space="PSUM") as psH,
            tc.tile_pool(name="psT", bufs=1, space="PSUM") as psT,
        ):
            # All startup-critical loads go on ONE queue (SP/sync) in
            # hand-chosen FIFO order: the first matmuls need aat+xt0, then
            # the W slabs pace tile 0, with xt1 slotted early. Later x tiles
            # and btx are triggered from the ACT stream *after* each tile's
            # sigmoid so they queue behind a data dependency and cannot
            # crowd the W stream.
            aat_sb = res.tile([P, KT * HN], bf16, tag="aat")
            btx_sb = res.tile([P, 512], bf16, tag="btx")
            brd128 = res.tile([P, 1], f32, tag="brd128")
            nc.gpsimd.dma_start(brd128[:], brd_d[:].broadcast_to((P, 1)))
            nc.gpsimd.dma_start(btx_sb[:], btx_d[:])

            xts = [None] * NT

            def load_x(t, eng, chunks=1):
                xts[t] = xpool.tile([P, KT * P], bf16, tag="xt", name=f"xt_{t}")
                cw = KT * P // chunks
                for cc in range(chunks):
                    eng.dma_start(xts[t][:, cc * cw:(cc + 1) * cw],
                                  xt_d[t * P:(t + 1) * P, cc * cw:(cc + 1) * cw])

            w_sb = res.tile([P, KT * O], bf16, tag="w_sb")

            def load_w(k):
                nc.sync.dma_start(w_sb[:, k * O:(k + 1) * O],
                                  wt_d[k * P:(k + 1) * P, :])

            # Sync-queue FIFO hand-ordering: the first h matmuls need aat +
            # xt0's first chunk only; W0 next so the main sweep can start;
            # xt1 early but behind the first W slabs. Everything else is
            # flow-controlled: xpool bufs=2 means tile t+2's load has a WAR
            # dependency on tile t's last read, so the scheduler cannot
            # hoist those transfers into the startup window.
            nc.sync.dma_start(aat_sb[:], aat_d[:])
            xts[0] = xpool.tile([P, KT * P], bf16, tag="xt", name="xt_0")
            nc.sync.dma_start(xts[0][:, 0:1024], xt_d[0:P, 0:1024])
            load_w(0)
            nc.sync.dma_start(xts[0][:, 1024:2048], xt_d[0:P, 1024:2048])
            load_w(1)
            load_w(2)
            load_x(1, nc.sync)
            for k in range(3, KT):
                load_w(k)

            ident = res.tile([P, P], bf16, tag="ident")
            masks.make_identity(nc, ident[:])

            # --- main loop: one 128-token tile at a time.
            # LoRA finalize for tile t is emitted after tile t's main sweep;
            # the h->sigmoid->g->g^T chain latency is mostly hidden because
            # the h matmul leads each k-group and the chain engines (ACT/DVE)
            # are otherwise idle.
            for t in range(NT):

                accs = [
                    psA.tile([P, 512], f32, tag="acc", name=f"acc_{t}_{j}")
                    for j in range(4)
                ]
                h = psH.tile([P, HN], f32, tag="h", name=f"h_{t}")
                gt4 = gpool.tile([P, P], bf16, tag="gt4", name=f"gt4_{t}")
                for k in range(KT):
                    lhs = xts[t][:, k * P:(k + 1) * P]
                    # h matmuls 2-per-iteration (front-loaded so the routing
                    # chain finishes during the main sweep), interleaved
                    # after the 512-col matmuls so their LDWEIGHTS hide.
                    for j in range(4):
                        nc.tensor.matmul(
                            accs[j][:], lhs,
                            w_sb[:, k * O + j * 512:k * O + (j + 1) * 512],
                            start=(k == 0), stop=False,
                        )
                        if k < 8 and j < 2:
                            c = 2 * k + j
                            nc.tensor.matmul(
                                h[:], xts[t][:, c * P:(c + 1) * P],
                                aat_sb[:, c * HN:(c + 1) * HN],
                                start=(c == 0), stop=(c == KT - 1))
                    # HAM warmers: while the PE trails the W^T DMA stream in
                    # tile 0 it would idle between k-groups and re-throttle to
                    # half clock; re-reading the just-arrived slab keeps it
                    # warm and costs nothing once W is resident.
                    if t == 0:
                        wdum = psT.tile([P, 512], f32, tag="tstage",
                                        name=f"wd_{k}")
                        nc.tensor.matmul(wdum[:], lhs,
                                         w_sb[:, k * O:k * O + 512],
                                         start=True, stop=True)
                    if k == 7:
                        # routing: w1 = sigmoid(dlogit + brd), w0 = 1 - w1,
                        # both scaled by alpha/rank
                        srow = small.tile([P, 1], f32, tag="srow",
                                          name=f"srow_{t}")
                        nc.scalar.activation(
                            srow[:], h[:, ER:ER + 1],
                            mybir.ActivationFunctionType.Sigmoid,
                            bias=brd128[:, 0:1], scale=1.0)
                        w1s = small.tile([P, 1], f32, tag="w1s", name=f"w1s_{t}")
                        nc.vector.tensor_scalar_mul(w1s[:], srow[:], SCALE)
                        w0s = small.tile([P, 1], f32, tag="w0s", name=f"w0s_{t}")
                        nc.vector.tensor_scalar(
                            w0s[:], srow[:], -SCALE, SCALE,
                            mybir.AluOpType.mult, mybir.AluOpType.add)
                        g = gpool.tile([P, GK], bf16, tag="g", name=f"g_{t}")
                        nc.vector.tensor_scalar_mul(g[:, 0:8], h[:, 0:8], w0s[:])
                        nc.vector.tensor_scalar_mul(g[:, 8:16], h[:, 8:16], w1s[:])
                        nc.vector.memset(g[:, 16:17], 1.0)
                        nc.vector.memset(g[:, 17:18], 0.0)
                    if k == 11:
                        # g^T on the PE, then replicate into 4 row strips so
                        # the finalize matmuls can run on 4 concurrent 32-row
                        # tiles of the array.
                        gst = psT.tile([GK, P], bf16, tag="tstage",
                                       name=f"gst_{t}")
                        nc.tensor.transpose(gst[:], g[:], ident[:])
                        for jj in range(4):
                            nc.scalar.copy(gt4[32 * jj:32 * jj + GK, :], gst[:])
                        # next-next x tile: xpool bufs=2 gives this a WAR
                        # dependency on tile t's last read, so the transfer
                        # paces itself behind the compute
                        if t + 2 < NT:
                            load_x(t + 2, nc.scalar)

                # LoRA finalize + bias: 4 concurrent row-tiled K=18 matmuls
                for jj in range(4):
                    nc.tensor.matmul(accs[jj][:],
                                     gt4[32 * jj:32 * jj + GK, :],
                                     btx_sb[32 * jj:32 * jj + GK, :],
                                     start=False, stop=True,
                                     tile_position=(32 * jj, 0))

                # drain: bias already folded in via the ones-row; pure copy,
                # split across Vector and Scalar engines.
                outt = opool.tile([P, O], bf16, tag="outt", name=f"out_{t}")
                for j in range(4):
                    if j < 2:
                        nc.vector.tensor_copy(outt[:, j * 512:(j + 1) * 512],
                                              accs[j][:])
                    else:
                        nc.scalar.copy(outt[:, j * 512:(j + 1) * 512],
                                       accs[j][:])
                    if j % 2 == 1:
                        nc.sync.dma_start(
                            out_d[t * P:(t + 1) * P, (j - 1) * 512:(j + 1) * 512],
                            outt[:, (j - 1) * 512:(j + 1) * 512])

    nc.compile()
    return nc


def _prep_host(x, W_base, b_base, A, B, W_router, b_router):
    """Host-side layout prep + sharding. Returns per-core input maps."""
    x_flat = np.ascontiguousarray(np.asarray(x, dtype=np.float32).reshape(-1, D))
    xb = x_flat.astype(BF16)
    # per-core pre-transposed tiling: [core, t, p(d within k), k*128+j(tok)]
    xt_all = np.ascontiguousarray(
        xb.reshape(NCORES, NT, P, KT, P).transpose(0, 1, 4, 3, 2)
    ).reshape(NCORES, NT * P, KT * P)

    wt = np.ascontiguousarray(np.asarray(W_base, dtype=np.float32).T).astype(BF16)

    a_cat = np.asarray(A, dtype=np.float32).reshape(ER, D)          # [16, D]
    aat = np.zeros((D, HN), dtype=np.float32)
    aat[:, :ER] = a_cat.T
    wr = np.asarray(W_router, dtype=np.float32)
    aat[:, ER] = wr[:, 1] - wr[:, 0]
    # pre-arrange for contiguous per-partition DMA: [P, KT*HN]
    aat = np.ascontiguousarray(
        aat.reshape(KT, P, HN).transpose(1, 0, 2).reshape(P, KT * HN)
    ).astype(BF16)

    b_cat = np.concatenate([np.asarray(B, dtype=np.float32)[0],
                            np.asarray(B, dtype=np.float32)[1]], axis=1)  # [O, 16]
    btx_full = np.zeros((GK, O), dtype=np.float32)
    btx_full[:ER] = b_cat.T
    btx_full[ER] = np.asarray(b_base, dtype=np.float32)  # ones-row adds bias
    # 4 row strips at partition offsets 0/32/64/96, one 512-col chunk each,
    # so the finalize matmuls run on 4 concurrent 32-row PE tiles
    btx = np.zeros((P, 512), dtype=np.float32)
    for jj in range(4):
        btx[32 * jj:32 * jj + GK] = btx_full[:, jj * 512:(jj + 1) * 512]
    btx = np.ascontiguousarray(btx).astype(BF16)

    brd = np.array([[np.float32(b_router[1]) - np.float32(b_router[0])]],
                   dtype=np.float32)

    in_maps = []
    for c in range(NCORES):
        in_maps.append({
            "xt": xt_all[c],
            "wt": wt,
            "aat": aat,
            "btx": btx,
            "brd": brd,
        })
    return in_maps


def kernel(x, W_base, b_base, A, B, W_router, b_router):
    from concourse import bass_utils

    # NOTE: walrus's LDWEIGHTS-dedup (--enable-ldw-opt=true) rejects bf16
    # weight loads ("InstLdweights is not compatible with LDW optimization")
    # because they take the FWL path; bf16 LDWs are ~2x faster than fp32 and
    # hidden by the PE's pull-ahead window, so the dedup isn't needed.
    if "nc" not in _CACHE:
        _CACHE["nc"] = _build()
    nc = _CACHE["nc"]

    in_maps = _prep_host(x, W_base, b_base, A, B, W_router, b_router)
    res = None
    for attempt in range(3):
        try:
            res = bass_utils.run_bass_kernel_spmd(
                nc, in_maps, core_ids=list(range(NCORES)))
            break
        except Exception:
            # rare transient NRT_EXEC_UNIT_UNRECOVERABLE observed once;
            # the same NEFF runs fine on retry
            if attempt == 2:
                raise
    out = np.concatenate(
        [np.asarray(res.results[c]["out"]).astype(np.float32)
         for c in range(NCORES)], axis=0)
    return out.reshape(np.asarray(x).shape[0], -1, O)
